# revision 15
# baseline (speedup 1.0000x reference)
"""Trainium2 Bass kernel for the CriticBaseline problem.

reference:
    G = discounted_returns(rewards)            # reverse scan, gamma=0.99
    h = relu(obs @ W1 + b1); h = relu(h @ W2 + b2)
    V = (h @ W3 + b3).reshape(-1)
    return G - V                               # [T]

Strategy (8 NeuronCores, SPMD, no collectives):
  - Data-parallel over T: core c owns timesteps [c*8192, (c+1)*8192).
  - The two big GEMMs run in fp8 e4m3 with perf_mode=DoubleRow: each
    matmul contracts K=256 (two 128-rows packed per PE cell) while
    streaming 512 moving columns -> ~2x the fp32r/bf16 ALU rate.
    Inputs are quantized host-side with power-of-2 scales (obs*16,
    W1*64, W2*64) so everything stays in e4m3 normal range; the scales
    fold into the activation's `scale` argument.  relu's positive
    homogeneity lets layer 1 emit h1 pre-scaled by 16 (fp8) so it can
    feed GEMM2 directly; layer 2 emits h2 unscaled in bf16 for the DVE
    layer-3 reduction.  fp8 quantization error lands on V, which is
    ~10x smaller than G, so the output rel-err stays ~5e-3 << 2e-2.
  - Layer 3 (h2 @ W3) is a bf16 per-partition scale on DVE, then the
    cross-partition sum is a single ones-vector matmul per 128-block
    (out[a, b] = sum_p acc[p, 128b+a] * 1), which lands V directly in
    the scan's [t%128, t/128] layout -- no PE transpose / DVE reduce.
  - The discounted-return scan is a banded bf16 matmul (gamma^k decays
    below 1.2e-9 by k=2048) -- 17 [128,128] coefficient matmuls.
  - All DRAM operands are host-packed so every DMA sees >=1KB
    contiguous runs per partition (the naive layouts fragment into
    128B/4B descriptors and stall the PE waiting on weights).
  - Each tile's output chunk is PE-transposed to [4, 128] so the store
    is 4 contiguous 512B lines instead of a 512x4B scatter (which cost
    an ~11us exposed tail).
  - A few dummy matmuls warm the PE's HAM clock gate (a cold PE runs at
    1.2 GHz for its first ~3.4us of activity) while the first obs/W1
    DMAs are still in flight; the startup DMAs are batched/ordered so
    the first GEMM can start ~13us in.
"""

import ml_dtypes
import numpy as np

GAMMA = 0.99
T, D, H = 65536, 1024, 1024
N_CORES = 8
TC = T // N_CORES  # 8192 timesteps per core
TT = 512           # moving-dim tile (one PSUM bank of fp32)
NT = TC // TT      # 16 t-tiles per core
NB = TC // 128     # 64 blocks of 128 timesteps
WIN = 2048         # scan window: gamma^2048 ~ 1.1e-9
NJ = WIN // 128    # 16 -> coefficient matrices j = 0..16
RCOLS = NB + NJ    # 80 columns of packed rewards per core
NWARM = 12         # PE warmup matmuls

SX = 16.0          # obs fp8 scale
SW = 64.0          # W1/W2 fp8 scale
SH = 16.0          # h1 fp8 scale
FP8 = ml_dtypes.float8_e4m3  # TRN e4m3: max normal 240
BF16 = ml_dtypes.bfloat16

_cache = {}


def _scan_mats() -> np.ndarray:
    """Mj[k, p] = gamma^(128j + k - p) on the band 0 <= 128j+k-p < WIN."""
    k = np.arange(128)[:, None]
    p = np.arange(128)[None, :]
    mats = []
    for j in range(NJ + 1):
        e = 128 * j + k - p
        m = np.where((e >= 0) & (e < WIN), np.power(GAMMA, e, dtype=np.float64), 0.0)
        mats.append(m.astype(np.float32))
    return np.ascontiguousarray(np.stack(mats))


def _q8(x: np.ndarray) -> np.ndarray:
    return np.clip(x, -240.0, 240.0).astype(FP8)


def _build():
    """Build + schedule the single-core SPMD Bass program (cached)."""
    if "nc" in _cache:
        return _cache["nc"]

    from contextlib import ExitStack

    import concourse.mybir as mybir
    import concourse.tile as tile
    from concourse import bacc
    from concourse.alu_op_type import AluOpType
    from concourse.masks import make_identity

    f32 = mybir.dt.float32
    bf16 = mybir.dt.bfloat16
    fp8 = mybir.dt.float8e4
    DR = mybir.MatmulPerfMode.DoubleRow
    Relu = mybir.ActivationFunctionType.Relu
    Copy = mybir.ActivationFunctionType.Copy

    nc = bacc.Bacc("TRN2", target_bir_lowering=False, debug=False, num_devices=N_CORES)

    # obs pre-packed host-side as [it, p, dk, i, t]: contiguous 4KB rows per tile
    obsq = nc.dram_tensor("obsq", [NT, 128, 4, 2, TT], fp8, kind="ExternalInput").ap()
    # W1 packed per output block: [ho, p, dk, i, m] -> 1KB contiguous rows
    w1q = nc.dram_tensor("w1q", [8, 128, 4, 2, 128], fp8, kind="ExternalInput").ap()
    w2q = nc.dram_tensor("w2q", [128, 4, 2, H], fp8, kind="ExternalInput").ap()
    # b1*SH | b2 | b3 | w3 packed into one tensor -> single startup DMA
    cst = nc.dram_tensor("cst", [128, 25], f32, kind="ExternalInput").ap()
    rmat = nc.dram_tensor("rmat", [128, RCOLS], bf16, kind="ExternalInput").ap()
    scanm = nc.dram_tensor("scanm", [128, NJ + 1, 128], bf16, kind="ExternalInput").ap()
    out = nc.dram_tensor("out", [TC], f32, kind="ExternalOutput").ap()
    outr = out.rearrange("(b p) -> b p", p=128)  # [NB, 128] row view

    with tile.TileContext(nc) as tc, ExitStack() as ctx:
        const = ctx.enter_context(tc.tile_pool(name="const", bufs=1))
        w1_sb = const.tile([128, 8, 4, 2, 128], fp8, name="w1_sb")
        w2_sb = const.tile([128, 4, 2, H], fp8, name="w2_sb")
        scan_sb = const.tile([128, NJ + 1, 128], bf16, name="scan_sb")
        rmat_sb = const.tile([128, RCOLS], bf16, name="rmat_sb")
        cst_sb = const.tile([128, 25], f32, name="cst_sb")
        b1_sb = cst_sb[:, 0:8]
        b2_sb = cst_sb[:, 8:16]
        b3_sb = cst_sb[:, 16:17]
        w3_sb = cst_sb[:, 17:25]
        ident = const.tile([128, 128], f32, name="ident")
        ones_sb = const.tile([128, 1], bf16, name="ones_sb")
        warm_src = const.tile([128, 512], bf16, name="warm_src")

        otp = ctx.enter_context(tc.tile_pool(name="otp", bufs=6))
        h1p = ctx.enter_context(tc.tile_pool(name="h1p", bufs=3))
        h2p = ctx.enter_context(tc.tile_pool(name="h2p", bufs=3))
        accp = ctx.enter_context(tc.tile_pool(name="accp", bufs=3))
        finp = ctx.enter_context(tc.tile_pool(name="finp", bufs=2))
        fintp = ctx.enter_context(tc.tile_pool(name="fintp", bufs=2))
        ps1 = ctx.enter_context(tc.tile_pool(name="ps1", bufs=3, space="PSUM"))
        ps2 = ctx.enter_context(tc.tile_pool(name="ps2", bufs=2, space="PSUM"))
        vtp = ctx.enter_context(tc.tile_pool(name="vtp", bufs=2, space="PSUM"))
        wps = ctx.enter_context(tc.tile_pool(name="wps", bufs=1, space="PSUM"))

        gsbp = ctx.enter_context(tc.tile_pool(name="gsbp", bufs=1))
        g_sb = gsbp.tile([128, NB], f32, name="g_sb")

        # ---- startup: tiny consts first (the Scalar engine's one-time
        # ACT_TABLE_LOAD waits on the bias DMAs -- queueing them behind the
        # big transfers stalled the first activations until ~21us), then
        # obs tile 0 + weights; PE warmup matmuls on the identity keep the
        # HAM clock gate busy while those DMAs land.
        nc.vector.memset(warm_src[:], 1.0)
        nc.gpsimd.memset(ones_sb[:], 1.0)
        ot0 = otp.tile([128, 4, 2, TT], fp8, tag="ot", name="ot_0")
        nc.sync.dma_start(ot0[:], obsq[0])
        nc.sync.dma_start(w1_sb[:, 0:2], w1q[0:2].rearrange("h p a b m -> p h a b m"))
        nc.sync.dma_start(cst_sb[:], cst[:])
        for hp in range(1, 4):
            nc.sync.dma_start(
                w1_sb[:, 2 * hp : 2 * hp + 2],
                w1q[2 * hp : 2 * hp + 2].rearrange("h p a b m -> p h a b m"),
            )
        nc.sync.dma_start(w2_sb[:], w2q[:])
        warm = wps.tile([128, 512], f32, tag="sm", name="warm")
        for i in range(NWARM):
            nc.tensor.matmul(
                warm[:, :], lhsT=warm_src[:, 0:128], rhs=warm_src[:], start=True, stop=True
            )
        make_identity(nc, ident[:])
        nc.sync.dma_start(scan_sb[:], scanm[:])
        nc.sync.dma_start(rmat_sb[:], rmat[:])

        for it in range(NT):
            if it == 0:
                ot = ot0
            else:
                ot = otp.tile([128, 4, 2, TT], fp8, tag="ot", name=f"ot_{it}")
                nc.sync.dma_start(ot[:], obsq[it])

            h1 = h1p.tile([128, 8, TT], fp8, tag="h1", name=f"h1_{it}")
            for ho in range(8):
                p1 = ps1.tile([128, TT], f32, tag="p1", name=f"p1_{it}_{ho}")
                for dk in range(4):
                    nc.tensor.matmul(
                        p1[:],
                        lhsT=w1_sb[:, ho, dk, :, :],
                        rhs=ot[:, dk, :, :],
                        start=(dk == 0),
                        stop=(dk == 3),
                        perf_mode=DR,
                    )
                # h1 = SH * relu(psum/(SX*SW) + b1) = relu(psum*SH/(SX*SW) + SH*b1)
                nc.scalar.activation(
                    h1[:, ho, :],
                    p1[:],
                    Relu,
                    bias=b1_sb[:, ho : ho + 1],
                    scale=SH / (SX * SW),
                )

            h2 = h2p.tile([128, 8, TT], bf16, tag="h2", name=f"h2_{it}")
            for ho in range(8):
                p2 = ps2.tile([128, TT], f32, tag="p2", name=f"p2_{it}_{ho}")
                for hk in range(4):
                    nc.tensor.matmul(
                        p2[:],
                        lhsT=w2_sb[:, hk, :, ho * 128 : (ho + 1) * 128],
                        rhs=h1[:, 2 * hk : 2 * hk + 2, :],
                        start=(hk == 0),
                        stop=(hk == 3),
                        perf_mode=DR,
                    )
                nc.scalar.activation(
                    h2[:, ho, :],
                    p2[:],
                    Relu,
                    bias=b2_sb[:, ho : ho + 1],
                    scale=1.0 / (SH * SW),
                )

            # V lane accumulation: acc[p, t] = sum_hj h2[128*hj+p, t] * W3[128*hj+p]
            acc = accp.tile([128, TT], bf16, tag="acc", name=f"acc_{it}")
            nc.vector.tensor_scalar_mul(acc[:], h2[:, 0, :], w3_sb[:, 0:1])
            for hj in range(1, 8):
                nc.vector.scalar_tensor_tensor(
                    acc[:],
                    h2[:, hj, :],
                    w3_sb[:, hj : hj + 1],
                    acc[:],
                    AluOpType.mult,
                    AluOpType.add,
                )
            # cross-partition reduce: one ones-vector matmul per 128-block
            # lands V[a, b] = sum_p acc[p, 128b+a] in [t%128, t/128] layout
            vt = vtp.tile([128, 4], f32, tag="vt", name=f"vt_{it}")
            for s in range(4):
                nc.tensor.matmul(
                    vt[:, s : s + 1],
                    lhsT=acc[:, s * 128 : (s + 1) * 128],
                    rhs=ones_sb[:],
                    start=(s == 0),
                    stop=(s == 3),
                    skip_group_check=(s != 0),
                )


            if it == 0:
                # discounted returns: 17 banded matmuls, bf16
                g_psum = wps.tile([128, NB], f32, tag="sm", name="g_psum")
                for j in range(NJ + 1):
                    nc.tensor.matmul(
                        g_psum[:],
                        lhsT=scan_sb[:, j, :],
                        rhs=rmat_sb[:, j : j + NB],
                        start=(j == 0),
                        stop=(j == NJ),
                    )
                nc.scalar.activation(g_sb[:], g_psum[:], Copy)

            # emit this tile's output: om = (G - b3) - V for 4 blocks,
            # PE-transposed to [4, 128] so the store is contiguous
            cs = slice(4 * it, 4 * (it + 1))
            om = finp.tile([128, 4], f32, tag="om", name=f"om_{it}")
            nc.vector.scalar_tensor_tensor(
                om[:],
                g_sb[:, cs],
                b3_sb[:, 0:1],
                vt[:],
                AluOpType.subtract,
                AluOpType.subtract,
            )
            omt = wps.tile([4, 128], f32, tag="sm", name=f"omt_{it}")
            nc.tensor.transpose(omt[:], om[:], ident[:])
            oms = fintp.tile([4, 128], f32, tag="oms", name=f"oms_{it}")
            nc.scalar.activation(oms[:], omt[:], Copy)
            nc.sync.dma_start(outr[cs, :], oms[:])

    nc.compile()
    _cache["nc"] = nc
    return nc


def _pack_inputs(rewards, obs, W1, b1, W2, b2, W3, b3):
    scanm = _scan_mats()  # [17, k, p]
    scanp = np.ascontiguousarray(scanm.transpose(1, 0, 2)).astype(BF16)  # [k, 17, p]
    cst = np.empty((128, 25), np.float32)
    cst[:, 0:8] = (SH * b1).reshape(8, 128).T
    cst[:, 8:16] = b2.reshape(8, 128).T
    cst[:, 16:17] = b3.reshape(1, 1)
    cst[:, 17:25] = W3.reshape(8, 128).T

    # weights packed with contraction index d = dk*256 + i*128 + p
    w1q = _q8(
        np.ascontiguousarray(
            (SW * W1).reshape(4, 2, 128, 8, 128).transpose(3, 2, 0, 1, 4)
        )
    )  # [ho, p, dk, i, m]
    w2q = _q8(np.ascontiguousarray((SW * W2).reshape(4, 2, 128, H).transpose(2, 0, 1, 3)))

    r_pad = np.zeros(T + WIN, dtype=np.float32)
    r_pad[:T] = rewards

    in_maps = []
    for c in range(N_CORES):
        lo = c * TC
        # obs chunk [Tc, D] -> fp8 [it, p, dk, i, s]
        oq = _q8(
            (SX * obs[lo : lo + TC]).reshape(NT, TT, 4, 2, 128).transpose(0, 4, 2, 3, 1)
        )
        in_maps.append(
            {
                "obsq": np.ascontiguousarray(oq),
                "w1q": w1q,
                "w2q": w2q,
                "cst": cst,
                "rmat": np.ascontiguousarray(
                    r_pad[lo : lo + TC + WIN].reshape(RCOLS, 128).T
                ).astype(BF16),
                "scanm": scanp,
            }
        )
    return in_maps


def kernel(rewards, obs, W1, b1, W2, b2, W3, b3):
    from concourse.bass_utils import run_bass_kernel_spmd

    rewards = np.asarray(rewards, dtype=np.float32)
    obs = np.asarray(obs, dtype=np.float32)
    W1 = np.ascontiguousarray(np.asarray(W1, dtype=np.float32))
    W2 = np.ascontiguousarray(np.asarray(W2, dtype=np.float32))
    W3 = np.asarray(W3, dtype=np.float32)
    b1 = np.asarray(b1, dtype=np.float32)
    b2 = np.asarray(b2, dtype=np.float32)
    b3 = np.asarray(b3, dtype=np.float32)

    nc = _build()
    in_maps = _pack_inputs(rewards, obs, W1, b1, W2, b2, W3, b3)
    res = run_bass_kernel_spmd(nc, in_maps, core_ids=list(range(N_CORES)))
    return np.concatenate([res.results[c]["out"] for c in range(N_CORES)])
